# revision 12
# baseline (speedup 1.0000x reference)
"""Trainium2 Bass kernel for nn_MeshDownConv (2-layer SplineConv GNN).

Sharding: 4 cores, one full mesh per core (zero cross-core communication).
The wall-clock of kernel() is dominated by host preprocessing and the
host<->device tunnel, so the design minimizes uploaded bytes AND transfer
count:

- ONE ~10.5 MB int16 blob per core (the tunnel has ~100 ms per-transfer
  overhead, so 4 blobs beat 36 tensors by ~3 s):
    tab0c [NN,32] f16   node features
    idxc  [32,NI16] i16 gather indices (one copy; device replicates x8 into
                        the 128-partition layout SWDGE wants)
    edd3  [128,NCHT,3]  per-edge-slot (dstoff, t0, t1); the 9 B-spline basis
                        values are computed on device in a prepass
    weights (f16)       converted to f32 on device where needed
- A device prepass expands these into internal-DRAM tensors (texp, idxT,
  edd10); the main loops are the incidence-matmul design: dma_gather source
  rows, DVE builds u[e,(k,c)] = basis_k*xj_c and the one-hot incidence, PE
  contracts edges into PSUM, node side multiplies by Wflat + root + bias +
  relu.
- Host prep is fully vectorized numpy writing straight into the blob.
- Execution uses a module-cached jit(shard_map) (repeat calls skip
  re-tracing / NEFF recompks); a persistent jax compilation cache makes
  fresh processes skip the NEFF compile too. Donated output buffers are
  created on device; output returns as f16 [NN,32] per core.
"""
import sys

sys.path.insert(0, "/opt/trn_rl_repo")

import numpy as np

import concourse.bass as bass
import concourse.mybir as mybir
from concourse import bacc, tile

F32 = mybir.dt.float32
F16 = mybir.dt.float16
I16 = mybir.dt.int16
I32 = mybir.dt.int32


class CFG:
    C = 32            # in channels
    O = 32            # out channels
    KK = 9            # spline kernels
    NW = 196          # windows per table half (i16 index range)
    NCHA = 9          # chunks (of 128 edges) per window per pass
    GW = 4            # windows per gather group
    SG = 8            # groups per idx-streaming supergroup
    NSPLIT = 2        # gather pieces per (group, pass)
    N = 50000         # real nodes per mesh
    E = 800000        # edges per mesh
    B = 4             # meshes
    NCORES = 4        # one mesh per core
    DT = F16

    @property
    def RL(self):
        return 64 if self.DT == F32 else 128   # 256B gather granule

    @property
    def NCH(self):
        return 2 * self.NCHA

    @property
    def NHALF(self):
        return self.NW * 128

    @property
    def NN(self):
        return 2 * self.NHALF

    @property
    def NWT(self):
        return 2 * self.NW            # windows per mesh (both layers)

    @property
    def NCHT(self):
        return self.NWT * self.NCH

    @property
    def NSLOTT(self):
        return self.NWT * self.NCHA * 128

    @property
    def NI16(self):
        return self.NSLOTT // 16

    # ---- blob layout (offsets in int16 elements, 128-elem aligned)
    @property
    def OFF_TAB(self):
        return 0

    @property
    def OFF_IDX(self):
        return self.OFF_TAB + self.NN * self.C

    @property
    def OFF_ED3(self):
        return self.OFF_IDX + 2 * self.NSLOTT

    @property
    def OFF_WF1(self):
        return self.OFF_ED3 + 128 * self.NCHT * 3

    @property
    def OFF_WF2(self):
        return self.OFF_WF1 + 96 * 96

    @property
    def OFF_RT1(self):
        return self.OFF_WF2 + 96 * 96

    @property
    def OFF_RT2(self):
        return self.OFF_RT1 + self.C * self.O

    @property
    def OFF_BB1(self):
        return self.OFF_RT2 + self.C * self.O

    @property
    def OFF_BB2(self):
        return self.OFF_BB1 + 128

    @property
    def TOT16(self):
        return self.OFF_BB2 + 128


def _np_dt(dt):
    return {F32: np.float32, F16: np.float16}[dt]


# ----------------------------------------------------------------- host prep

def _snake_sigma(deg, nbins, cap):
    """Balanced node->slot permutation: sort by degree desc, snake over bins.
    sigma[node] = bin*cap + slot."""
    n = deg.shape[0]
    order = np.argsort(-deg, kind="stable")
    r = np.arange(n, dtype=np.int64)
    row = r // nbins
    col = r % nbins
    binidx = np.where(row % 2 == 0, col, nbins - 1 - col)
    sigma = np.empty(n, np.int64)
    sigma[order] = binidx * cap + row
    return sigma


def _balance_nodes_greedy(deg, nbins, cap_nodes=128):
    import heapq
    n = deg.shape[0]
    order = np.argsort(-deg, kind="stable")
    heap = [(0, b) for b in range(nbins)]
    heapq.heapify(heap)
    counts = np.zeros(nbins, np.int64)
    sums = np.zeros(nbins, np.int64)
    sigma = np.empty(n, np.int64)
    for old in order:
        while True:
            s, b = heapq.heappop(heap)
            if counts[b] < cap_nodes:
                break
        sigma[old] = b * 128 + counts[b]
        counts[b] += 1
        sums[b] += deg[old]
        if counts[b] < cap_nodes:
            heapq.heappush(heap, (sums[b], b))
    return sigma


def _prep_worker(args):
    """Process-pool entry: prep one mesh, return (blob, sigma)."""
    x, edge, pseudo = args
    cfg = CFG()
    blob = np.empty(cfg.TOT16, np.int16)
    tab0c = blob[cfg.OFF_TAB:cfg.OFF_IDX].view(np.float16).reshape(
        cfg.NN, cfg.C)
    idxsec = blob[cfg.OFF_IDX:cfg.OFF_ED3]
    ED3 = blob[cfg.OFF_ED3:cfg.OFF_WF1].view(np.float16).reshape(
        128, cfg.NCHT, 3)
    sigma = _host_prep_mesh(cfg, np.asarray(x, np.float32),
                            np.asarray(edge), np.asarray(pseudo, np.float32),
                            tab0c, idxsec, ED3)
    return blob, sigma


def _host_prep_mesh(cfg, x, edge, pseudo, tab0c, idxsec, ED3):
    """Vectorized per-mesh host preprocessing, writing into blob views:
    tab0c [NN,32] f16, idxsec [2*NSLOTT] i16, ED3 [128,NCHT,3] f16.
    Returns sigma (node->row permutation)."""
    npdt = _np_dt(cfg.DT)
    src = np.ascontiguousarray(edge[0], np.int32)
    dst = np.ascontiguousarray(edge[1], np.int32)
    E = src.shape[0]
    capa = cfg.NCHA * 128

    deg = np.bincount(dst, minlength=cfg.N)
    for attempt in range(2):
        if attempt == 0:
            sigma = _snake_sigma(deg, cfg.NWT, 128).astype(np.int32)
        else:
            sigma = _balance_nodes_greedy(deg, cfg.NWT).astype(np.int32)
        gsrc = sigma[src]
        gdst = sigma[dst]
        # key = window(dst)*2 + srcpass fits int16 (< 2*NWT = 784)
        key = ((gdst >> 7) * 2 + (gsrc >= cfg.NHALF)).astype(np.int16)
        order = np.argsort(key, kind="stable")   # radix on int16
        sk = key[order].astype(np.int32)
        starts = np.searchsorted(sk, np.arange(2 * cfg.NWT + 1)).astype(
            np.int32)
        slot = np.arange(E, dtype=np.int32) - starts[sk]
        if slot.max() < capa:
            break
    else:
        raise RuntimeError("window capacity exceeded")

    w = sk >> 1
    pp = sk & 1

    # per-slot edge data; precompute f16 source columns, then gather
    dlow = (gdst & 127).astype(npdt)
    ps16 = pseudo.astype(npdt)
    ED3[:, :, 0] = 128.0          # sentinel: empty slots match no node lane
    ED3[:, :, 1:] = 0
    ev = np.empty((E, 3), npdt)
    ev[:, 0] = dlow[order]
    ev[:, 1:] = ps16[order]
    ch = pp * cfg.NCHA + w * cfg.NCH + (slot >> 7)
    ED3[slot & 127, ch] = ev

    # gather indices: linear per-pass layout, then reshape-transpose into
    # the 16-partition-wrapped layout SWDGE wants
    gslot = w * capa + slot
    vals = (gsrc[order] - pp * cfg.NHALF).astype(np.int16)
    IDXL = np.zeros((2, cfg.NSLOTT), np.int16)
    IDXL[pp, gslot] = vals
    idxsec.reshape(2, 16, cfg.NI16)[:] = IDXL.reshape(
        2, cfg.NI16, 16).transpose(0, 2, 1)

    tab0c[:] = 0
    tab0c[sigma] = x.astype(npdt)
    return sigma


# ------------------------------------------------------------- bass program

def _ap(t, offset, pattern):
    base = t if isinstance(t, bass.AP) else t[:]
    return bass.AP(base.tensor, base.offset + offset, pattern)


def _sap(t, offset, freedims, npart=None):
    base = t if isinstance(t, bass.AP) else t[:]
    p = base.ap[0]
    part = [p[0], p[1] if npart is None else npart]
    return bass.AP(base.tensor, base.offset + offset, [part] + list(freedims))


def build_program(cfg, ncores):
    nc = bacc.Bacc("TRN2", target_bir_lowering=False, debug=False,
                   num_devices=ncores)
    DT = cfg.DT
    C, O, KK, RL = cfg.C, cfg.O, cfg.KK, cfg.RL
    NCH, NCHA, GW = cfg.NCH, cfg.NCHA, cfg.GW
    NI16, NCHT, NN, NHALF = cfg.NI16, cfg.NCHT, cfg.NN, cfg.NHALF
    NG = cfg.NWT // GW            # 98 groups cover the whole mesh
    UD = KK * C                   # 288

    blob = nc.dram_tensor("blob", [cfg.TOT16], I16, kind="ExternalInput")
    outt = nc.dram_tensor("out", [NN, C], DT, kind="ExternalOutput")

    bi16 = blob.ap()
    bf16 = bi16.bitcast(F16)

    def bview(base, off, pattern):
        return bass.AP(base.tensor, off, pattern)

    with tile.TileContext(nc, num_cores=ncores) as tc:
        with tc.tile_pool(name="const", bufs=1) as cpool, \
             tc.tile_pool(name="dram", bufs=1, space="DRAM") as dpool, \
             tc.tile_pool(name="work", bufs=2) as wpool, \
             tc.tile_pool(name="psum", bufs=1, space="PSUM") as ppool:

            from concourse import library_config
            nc.gpsimd.load_library(library_config.mlp)

            # ---- constants
            iotai = cpool.tile([128, 128], I32, name="iotai")
            nc.gpsimd.iota(iotai[:], pattern=[[1, 128]], base=0,
                           channel_multiplier=0)
            iotaf = cpool.tile([128, 128], DT, name="iotaf")
            nc.vector.tensor_copy(iotaf[:], iotai[:])
            idximp = cpool.tile([128, 128], I32, name="idximp")
            nc.gpsimd.iota(idximp[:], pattern=[[1, 128]], base=0,
                           channel_multiplier=-1)
            identf = cpool.tile([128, 128], F32, name="identf")
            nc.vector.tensor_scalar(out=identf[:], in0=idximp[:],
                                    scalar1=0, scalar2=None,
                                    op0=mybir.AluOpType.is_equal)
            identd = identf
            if DT != F32:
                identd = cpool.tile([128, 128], DT, name="identd")
                nc.vector.tensor_copy(identd[:], identf[:])

            # weights: f16 in the blob, converted to f32 tiles where the
            # node-side matmul accumulates in f32
            wfs, rts, bbs = [], [], []
            for i, (owf, ort, obb) in enumerate(
                    [(cfg.OFF_WF1, cfg.OFF_RT1, cfg.OFF_BB1),
                     (cfg.OFF_WF2, cfg.OFF_RT2, cfg.OFF_BB2)]):
                wf16 = cpool.tile([96, 96], DT, name=f"wf16_{i}")
                nc.sync.dma_start(
                    wf16[:], bview(bf16, owf, [[96, 96], [1, 96]]))
                wf32 = cpool.tile([96, 96], F32, name=f"wfs{i}")
                nc.vector.tensor_copy(wf32[:], wf16[:])
                wfs.append(wf32)
                rt = cpool.tile([C, O], DT, name=f"rts{i}")
                nc.sync.dma_start(
                    rt[:], bview(bf16, ort, [[O, C], [1, O]]))
                rts.append(rt)
                bb16 = cpool.tile([O, 1], DT, name=f"bb16_{i}")
                nc.sync.dma_start(
                    bb16[:], bview(bf16, obb, [[1, O], [1, 1]]))
                bb32 = cpool.tile([O, 1], F32, name=f"bbs{i}")
                nc.vector.tensor_copy(bb32[:], bb16[:])
                bbs.append(bb32)

            # ---- internal DRAM
            texp = dpool.tile([NN, RL], DT, name="texp")
            tfull = dpool.tile([NN, RL], DT, name="tfull")
            idxT = dpool.tile([128, 2 * NI16], I16, name="idxT")
            edd10 = dpool.tile([128, NCHT * 10], DT, name="edd10")

            # ---- prepass 1: expand the node table to the 256B granule
            nc.sync.dma_start(
                _ap(texp, 0, [[RL, NN], [1, C]]),
                bview(bf16, cfg.OFF_TAB, [[C, NN], [1, C]]))

            # ---- prepass 2: replicate gather indices x8 into 128 partitions
            for pp in range(2):
                for k in range(8):
                    nc.sync.dma_start(
                        _ap(idxT, k * 16 * (2 * NI16) + pp * NI16,
                            [[2 * NI16, 16], [1, NI16]]),
                        bview(bi16, cfg.OFF_IDX + pp * 16 * NI16,
                              [[NI16, 16], [1, NI16]]))

            # ---- prepass 3: expand (dstoff,t0,t1) -> (dstoff, 9 basis vals)
            SQH = float(np.sqrt(0.5))
            bias_sq = cpool.tile([128, 1], F32, name="bias_sq")
            nc.vector.memset(bias_sq[:], SQH)
            bias_z = cpool.tile([128, 1], F32, name="bias_z")
            nc.vector.memset(bias_z[:], 0.0)
            NCHUNK = 16
            CC = NCHT // NCHUNK
            for ci in range(NCHUNK):
                e3 = wpool.tile([128, CC * 3], DT, name="pe3", bufs=2)
                nc.sync.dma_start(
                    e3[:], bview(bf16, cfg.OFF_ED3 + ci * CC * 3,
                                 [[NCHT * 3, 128], [1, CC * 3]]))
                bts = []
                for d in range(2):
                    bt = wpool.tile([128, CC * 3], DT, name=f"pb{d}", bufs=2)
                    tv = _sap(e3, 1 + d, [[3, CC]])
                    # open quadratic B-spline pieces of t in [0,1):
                    # B0 = 0.5(1-t)^2, B2 = 0.5 t^2, B1 = 1 - B0 - B2
                    nc.scalar.activation(
                        _sap(bt, 0, [[3, CC]]), tv,
                        mybir.ActivationFunctionType.Square,
                        bias=bias_sq[:], scale=-SQH)
                    nc.scalar.activation(
                        _sap(bt, 2, [[3, CC]]), tv,
                        mybir.ActivationFunctionType.Square,
                        bias=bias_z[:], scale=SQH)
                    tmp = wpool.tile([128, CC], DT, name=f"pt{d}", bufs=2)
                    nc.vector.tensor_tensor(
                        out=tmp[:], in0=_sap(bt, 0, [[3, CC]]),
                        in1=_sap(bt, 2, [[3, CC]]),
                        op=mybir.AluOpType.add)
                    nc.vector.tensor_scalar(
                        out=_sap(bt, 1, [[3, CC]]), in0=tmp[:],
                        scalar1=-1.0, scalar2=1.0,
                        op0=mybir.AluOpType.mult,
                        op1=mybir.AluOpType.add)
                    bts.append(bt)
                e10 = wpool.tile([128, CC * 10], DT, name="pe10", bufs=2)
                nc.vector.tensor_copy(_sap(e10, 0, [[10, CC]]),
                                      _sap(e3, 0, [[3, CC]]))
                # basis[c, 3j+i] = B1[c,j] * B0[c,i]
                nc.vector.tensor_tensor(
                    out=_sap(e10, 1, [[10, CC], [3, 3], [1, 3]]),
                    in0=_sap(bts[0], 0, [[3, CC], [0, 3], [1, 3]]),
                    in1=_sap(bts[1], 0, [[3, CC], [1, 3], [0, 3]]),
                    op=mybir.AluOpType.mult)
                nc.sync.dma_start(
                    _ap(edd10, ci * CC * 10, [[NCHT * 10, 128], [1, CC * 10]]),
                    e10[:])

            nsg = GW * NCHA * 128          # gather idxs per (group, pass)
            npiece = nsg // cfg.NSPLIT
            nchp = GW * NCHA // cfg.NSPLIT  # gathered chunks per piece

            def layer(tabsrc, xown, xrl, wfsb, rtsb, bbsb, rows_out, orl,
                      last):
                for g0 in range(0, NG, cfg.SG):
                    gcnt = min(cfg.SG, NG - g0)
                    sidxt = [None, None]
                    for pdx in range(2):
                        st = wpool.tile([128, cfg.SG * nsg // 16], I16,
                                        name=f"sidx{pdx}", bufs=2)
                        nc.sync.dma_start(
                            _sap(st, 0, [[1, gcnt * nsg // 16]]),
                            _ap(idxT, pdx * NI16 + g0 * nsg // 16,
                                [[2 * NI16, 128], [1, gcnt * nsg // 16]]))
                        sidxt[pdx] = st
                    for gl in range(gcnt):
                        g = g0 + gl
                        xjt = []
                        for pdx in range(2):
                            xj = wpool.tile([128, GW * NCHA * RL], DT,
                                            name=f"xj{pdx}", bufs=2)
                            for s in range(cfg.NSPLIT):
                                nc.gpsimd.dma_gather(
                                    out_ap=_sap(xj, s * nchp * RL,
                                                [[RL, nchp], [1, RL]]),
                                    in_ap=_ap(tabsrc, pdx * NHALF * RL,
                                              [[RL, NHALF], [1, RL]]),
                                    idxs_ap=_sap(
                                        sidxt[pdx],
                                        (gl * nsg + s * npiece) // 16,
                                        [[1, npiece // 16]]),
                                    num_idxs=npiece,
                                    num_idxs_reg=npiece,
                                    elem_size=RL,
                                    single_packet=False,
                                )
                            xjt.append(xj)
                        edt = wpool.tile([128, GW * NCH * 10], DT,
                                         name="edt", bufs=2)
                        nc.sync.dma_start(
                            edt[:],
                            _ap(edd10, g * GW * NCH * 10,
                                [[NCHT * 10, 128], [1, GW * NCH * 10]]))
                        xwing = wpool.tile([128, GW * C], DT, name="xwing",
                                           bufs=2)
                        nc.sync.dma_start(
                            xwing[:],
                            _ap(xown, g * GW * 128 * xrl,
                                [[xrl, 128], [128 * xrl, GW], [1, C]]))
                        rowsg = wpool.tile([128, GW * C], DT, name="rowsg",
                                           bufs=2)

                        for wl in range(GW):
                            u = wpool.tile([128, NCH * UD], DT, name="u",
                                           bufs=2)
                            for pdx in range(2):
                                nc.vector.tensor_tensor(
                                    out=_sap(u, pdx * NCHA * UD,
                                             [[UD, NCHA], [C, KK], [1, C]]),
                                    in0=_sap(xjt[pdx], wl * NCHA * RL,
                                             [[RL, NCHA], [0, KK], [1, C]]),
                                    in1=_sap(edt,
                                             (wl * NCH + pdx * NCHA) * 10 + 1,
                                             [[10, NCHA], [1, KK], [0, C]]),
                                    op=mybir.AluOpType.mult)
                            inc = wpool.tile([128, NCH * 128], DT,
                                             name="inc", bufs=2)
                            nc.vector.tensor_tensor(
                                out=_sap(inc, 0, [[128, NCH], [1, 128]]),
                                in0=_sap(iotaf, 0, [[0, NCH], [1, 128]]),
                                in1=_sap(edt, wl * NCH * 10,
                                         [[10, NCH], [0, 128]]),
                                op=mybir.AluOpType.is_equal)

                            z = ppool.tile([128, UD], F32, name="z", bufs=2)
                            for c in range(NCH):
                                nc.tensor.matmul(
                                    z[:],
                                    _sap(inc, c * 128, [[1, 128]]),
                                    _sap(u, c * UD, [[1, UD]]),
                                    start=(c == 0), stop=(c == NCH - 1))

                            zsb = wpool.tile([128, UD], F32, name="zsb",
                                             bufs=2)
                            nc.scalar.copy(zsb[:], z[:])
                            zt = ppool.tile([96, 384], F32, name="zt",
                                            bufs=2)
                            for j in range(3):
                                nc.tensor.transpose(
                                    _sap(zt, j * 128, [[1, 128]]),
                                    _sap(zsb, j * 96, [[1, 96]]),
                                    identf[:])
                            ztsb = wpool.tile([96, 384], F32, name="ztsb",
                                              bufs=2)
                            nc.scalar.copy(ztsb[:], zt[:])

                            agg = ppool.tile([O, 128], F32, name="agg",
                                             bufs=1)
                            for j in range(3):
                                nc.tensor.matmul(
                                    agg[:],
                                    _sap(wfsb, j * 32, [[1, 32]]),
                                    _sap(ztsb, j * 128, [[1, 128]]),
                                    start=(j == 0), stop=False)
                            xt = ppool.tile([C, 128], DT, name="xt", bufs=1)
                            nc.tensor.transpose(
                                xt[:],
                                _sap(xwing, wl * C, [[1, C]]),
                                identd[:])
                            xtsb = wpool.tile([C, 128], DT, name="xtsb",
                                              bufs=2)
                            nc.scalar.copy(xtsb[:], xt[:])
                            nc.tensor.matmul(agg[:], rtsb[:], xtsb[:],
                                             start=False, stop=True)
                            ht = wpool.tile([O, 128], DT, name="ht",
                                            bufs=2)
                            nc.scalar.activation(
                                ht[:], agg[:],
                                mybir.ActivationFunctionType.Relu,
                                bias=bbsb[:], scale=1.0)
                            rows = ppool.tile([128, O], DT, name="rows",
                                              bufs=1)
                            nc.tensor.transpose(
                                rows[:], ht[:],
                                _sap(identd, 0, [[1, 32]], npart=32))
                            nc.scalar.copy(
                                _sap(rowsg, wl * C, [[1, C]]), rows[:])

                        nc.sync.dma_start(
                            _ap(rows_out, g * GW * 128 * orl,
                                [[orl, 128], [128 * orl, GW], [1, C]]),
                            rowsg[:])

            # layer 1: gathers from texp, root term from the compact blob
            # table, writes the local full table
            layer(texp[:], bview(bf16, cfg.OFF_TAB, [[1, 1]]), C,
                  wfs[0], rts[0], bbs[0], tfull[:], RL, last=False)
            # layer 2: gathers from tfull, writes the compact f16 output
            layer(tfull[:], tfull[:], RL, wfs[1], rts[1], bbs[1],
                  outt.ap(), C, last=True)

    nc.finalize()
    return nc


# ------------------------------------------------------------------- runner

_RT = None


def _get_runtime(cfg):
    global _RT
    if _RT is not None:
        return _RT

    import jax
    import jax.numpy as jnp
    from jax.sharding import Mesh, PartitionSpec, NamedSharding
    from jax.experimental.shard_map import shard_map
    from concourse.bass2jax import (_bass_exec_p, install_neuronx_cc_hook,
                                    partition_id_tensor)

    try:
        jax.config.update("jax_compilation_cache_dir", "/tmp/meshconv_jaxcache")
        jax.config.update("jax_persistent_cache_min_compile_time_secs", 0.5)
    except Exception:
        pass

    # spawn the prep workers first so their interpreter startup overlaps
    # the (slow) program build below
    pool = None
    try:
        import multiprocessing as mp
        pool = mp.get_context("spawn").Pool(cfg.B)
    except Exception:
        pool = None

    nc = build_program(cfg, cfg.NCORES)
    install_neuronx_cc_hook()

    partition_name = (nc.partition_id_tensor.name
                      if nc.partition_id_tensor else None)
    in_names, out_names, out_avals = [], [], []
    for alloc in nc.m.functions[0].allocations:
        if not isinstance(alloc, mybir.MemoryLocationSet):
            continue
        name = alloc.memorylocations[0].name
        if alloc.kind == "ExternalInput":
            if name != partition_name:
                in_names.append(name)
        elif alloc.kind == "ExternalOutput":
            out_names.append(name)
            out_avals.append(jax.core.ShapedArray(
                tuple(alloc.tensor_shape), mybir.dt.np(alloc.dtype)))
    n_params = len(in_names)
    n_outs = len(out_names)
    all_names = list(in_names) + list(out_names)
    if partition_name is not None:
        all_names.append(partition_name)
    donate = tuple(range(n_params, n_params + n_outs))

    n_cores = cfg.NCORES
    devices = jax.devices()[:n_cores]
    mesh = Mesh(np.asarray(devices), ("core",))
    spec = NamedSharding(mesh, PartitionSpec("core"))

    def _body(*args):
        operands = list(args)
        if partition_name is not None:
            operands.append(partition_id_tensor())
        outs = _bass_exec_p.bind(
            *operands,
            out_avals=tuple(out_avals),
            in_names=tuple(all_names),
            out_names=tuple(out_names),
            lowering_input_output_aliases=(),
            sim_require_finite=True,
            sim_require_nnan=True,
            nc=nc,
        )
        return tuple(outs)

    sharded = jax.jit(
        shard_map(_body, mesh=mesh,
                  in_specs=(PartitionSpec("core"),) * (n_params + n_outs),
                  out_specs=(PartitionSpec("core"),) * n_outs,
                  check_rep=False),
        donate_argnums=donate, keep_unused=True)

    def _zeros():
        return tuple(
            jnp.zeros((n_cores * a.shape[0], *a.shape[1:]), a.dtype)
            for a in out_avals)
    zeros_fn = jax.jit(_zeros, out_shardings=(spec,) * n_outs)

    _RT = dict(nc=nc, jax=jax, mesh=mesh, spec=spec, devices=devices,
               in_names=in_names, out_names=out_names, out_avals=out_avals,
               sharded=sharded, zeros_fn=zeros_fn, pool=pool)
    return _RT


def _const16(cfg, W1, root1, b1, W2, root2, b2):
    """Pack weights into the int16 blob tail (f16 payloads)."""
    sec = np.zeros(cfg.TOT16 - cfg.OFF_WF1, np.int16)

    def put(off, arr16):
        v = arr16.view(np.int16).ravel()
        sec[off - cfg.OFF_WF1:off - cfg.OFF_WF1 + v.size] = v

    for W, off in ((W1, cfg.OFF_WF1), (W2, cfg.OFF_WF2)):
        Wflat = np.asarray(W, np.float32).reshape(cfg.KK * cfg.C, cfg.O)
        wfl = np.zeros((96, 96), np.float32)
        for j in range(3):
            wfl[:, 32 * j:32 * j + 32] = Wflat[96 * j:96 * j + 96, :]
        put(off, wfl.astype(np.float16))
    put(cfg.OFF_RT1, np.asarray(root1, np.float32).astype(np.float16))
    put(cfg.OFF_RT2, np.asarray(root2, np.float32).astype(np.float16))
    put(cfg.OFF_BB1, np.asarray(b1, np.float32).astype(np.float16))
    put(cfg.OFF_BB2, np.asarray(b2, np.float32).astype(np.float16))
    return sec


def run(cfg, images, edges, pseudo, W1, root1, b1, W2, root2, b2,
        trace=False, trace_out=None):
    rt = _get_runtime(cfg)
    jax = rt["jax"]
    devices = rt["devices"]

    csec = _const16(cfg, W1, root1, b1, W2, root2, b2)

    # host prep per mesh straight into the blob; device_put (async) each
    # blob as soon as it is ready so uploads overlap later meshes' prep
    shards = [None] * cfg.NCORES
    sigmas = [None] * cfg.B
    done = False
    if rt["pool"] is not None:
        try:
            futs = [rt["pool"].apply_async(
                _prep_worker, ((images[b], edges[b], pseudo[b]),))
                for b in range(cfg.B)]
            for b in range(cfg.B):
                blob, sigma = futs[b].get(120)
                blob[cfg.OFF_WF1:] = csec
                sigmas[b] = sigma
                shards[b] = jax.device_put(blob, devices[b])
            done = True
        except Exception:
            done = False
    if not done:
        for b in range(cfg.B):
            blob = np.empty(cfg.TOT16, np.int16)
            tab0c = blob[cfg.OFF_TAB:cfg.OFF_IDX].view(np.float16).reshape(
                cfg.NN, cfg.C)
            idxsec = blob[cfg.OFF_IDX:cfg.OFF_ED3]
            ED3 = blob[cfg.OFF_ED3:cfg.OFF_WF1].view(np.float16).reshape(
                128, cfg.NCHT, 3)
            sigma = _host_prep_mesh(
                cfg, np.asarray(images[b], np.float32),
                np.asarray(edges[b]), np.asarray(pseudo[b], np.float32),
                tab0c, idxsec, ED3)
            blob[cfg.OFF_WF1:] = csec
            sigmas[b] = sigma
            shards[b] = jax.device_put(blob, devices[b])

    garr = jax.make_array_from_single_device_arrays(
        (cfg.NCORES * cfg.TOT16,), rt["spec"], shards)
    zeros = rt["zeros_fn"]()

    out_arrs = rt["sharded"](garr, *zeros)
    outg = np.asarray(out_arrs[0]).reshape(cfg.NCORES, cfg.NN, cfg.C)

    out = np.empty((cfg.B, cfg.N, cfg.O), np.float32)
    for b in range(cfg.B):
        out[b] = outg[b].astype(np.float32)[sigmas[b]]
    return out


def kernel(images, edges, pseudo, W1, root1, b1, W2, root2, b2):
    cfg = CFG()
    return run(cfg, images, edges, pseudo, W1, root1, b1,
               W2, root2, b2)


# revision 15
# speedup vs baseline: 81.5129x; 81.5129x over previous
"""Trainium2 Bass kernel for nn_MeshDownConv (2-layer SplineConv GNN).

Sharding: 4 cores, one full mesh per core (zero cross-core communication).
The wall-clock of kernel() is dominated by host preprocessing and the
host<->device tunnel, so the design minimizes uploaded bytes AND transfer
count:

- ONE ~10.5 MB int16 blob per core (the tunnel has ~100 ms per-transfer
  overhead, so 4 blobs beat 36 tensors by ~3 s):
    tab0c [NN,32] f16   node features
    idxc  [32,NI16] i16 gather indices (one copy; device replicates x8 into
                        the 128-partition layout SWDGE wants)
    edd3  [128,NCHT,3]  per-edge-slot (dstoff, t0, t1); the 9 B-spline basis
                        values are computed on device in a prepass
    weights (f16)       converted to f32 on device where needed
- A device prepass expands these into internal-DRAM tensors (texp, idxT,
  edd10); the main loops are the incidence-matmul design: dma_gather source
  rows, DVE builds u[e,(k,c)] = basis_k*xj_c and the one-hot incidence, PE
  contracts edges into PSUM, node side multiplies by Wflat + root + bias +
  relu.
- Host prep is fully vectorized numpy writing straight into the blob.
- Execution uses a module-cached jit(shard_map) (repeat calls skip
  re-tracing / NEFF recompks); a persistent jax compilation cache makes
  fresh processes skip the NEFF compile too. Donated output buffers are
  created on device; output returns as f16 [NN,32] per core.
"""
import sys

sys.path.insert(0, "/opt/trn_rl_repo")

import numpy as np

import concourse.bass as bass
import concourse.mybir as mybir
from concourse import bacc, tile

F32 = mybir.dt.float32
F16 = mybir.dt.float16
I16 = mybir.dt.int16
I32 = mybir.dt.int32


class CFG:
    C = 32            # in channels
    O = 32            # out channels
    KK = 9            # spline kernels
    NW = 196          # windows per table half (i16 index range)
    NCHA = 9          # chunks (of 128 edges) per window per pass
    GW = 4            # windows per gather group
    SG = 8            # groups per idx-streaming supergroup
    NSPLIT = 2        # gather pieces per (group, pass)
    N = 50000         # real nodes per mesh
    E = 800000        # edges per mesh
    B = 4             # meshes
    NCORES = 4        # one mesh per core
    DT = F16

    @property
    def RL(self):
        return 64 if self.DT == F32 else 128   # 256B gather granule

    @property
    def NCH(self):
        return 2 * self.NCHA

    @property
    def NHALF(self):
        return self.NW * 128

    @property
    def NN(self):
        return 2 * self.NHALF

    @property
    def NWT(self):
        return 2 * self.NW            # windows per mesh (both layers)

    @property
    def NCHT(self):
        return self.NWT * self.NCH

    @property
    def NSLOTT(self):
        return self.NWT * self.NCHA * 128

    @property
    def NI16(self):
        return self.NSLOTT // 16

    # ---- blob layout (offsets in int16 elements, 128-elem aligned)
    @property
    def OFF_TAB(self):
        return 0

    @property
    def OFF_IDX(self):
        return self.OFF_TAB + self.NN * self.C

    @property
    def OFF_ED3(self):
        return self.OFF_IDX + 2 * self.NSLOTT

    @property
    def OFF_WF1(self):
        return self.OFF_ED3 + 128 * self.NCHT * 3

    @property
    def OFF_WF2(self):
        return self.OFF_WF1 + 96 * 96

    @property
    def OFF_RT1(self):
        return self.OFF_WF2 + 96 * 96

    @property
    def OFF_RT2(self):
        return self.OFF_RT1 + self.C * self.O

    @property
    def OFF_BB1(self):
        return self.OFF_RT2 + self.C * self.O

    @property
    def OFF_BB2(self):
        return self.OFF_BB1 + 128

    @property
    def TOT16(self):
        return self.OFF_BB2 + 128


def _np_dt(dt):
    return {F32: np.float32, F16: np.float16}[dt]


# ----------------------------------------------------------------- host prep

def _snake_sigma(deg, nbins, cap):
    """Balanced node->slot permutation: sort by degree desc, snake over bins.
    sigma[node] = bin*cap + slot."""
    n = deg.shape[0]
    order = np.argsort(-deg, kind="stable")
    r = np.arange(n, dtype=np.int64)
    row = r // nbins
    col = r % nbins
    binidx = np.where(row % 2 == 0, col, nbins - 1 - col)
    sigma = np.empty(n, np.int64)
    sigma[order] = binidx * cap + row
    return sigma


def _balance_nodes_greedy(deg, nbins, cap_nodes=128):
    import heapq
    n = deg.shape[0]
    order = np.argsort(-deg, kind="stable")
    heap = [(0, b) for b in range(nbins)]
    heapq.heapify(heap)
    counts = np.zeros(nbins, np.int64)
    sums = np.zeros(nbins, np.int64)
    sigma = np.empty(n, np.int64)
    for old in order:
        while True:
            s, b = heapq.heappop(heap)
            if counts[b] < cap_nodes:
                break
        sigma[old] = b * 128 + counts[b]
        counts[b] += 1
        sums[b] += deg[old]
        if counts[b] < cap_nodes:
            heapq.heappush(heap, (sums[b], b))
    return sigma


def _prep_worker(args):
    """Process-pool entry: prep one mesh, return (blob, sigma)."""
    x, edge, pseudo = args
    cfg = CFG()
    blob = np.empty(cfg.TOT16, np.int16)
    tab0c = blob[cfg.OFF_TAB:cfg.OFF_IDX].view(np.float16).reshape(
        cfg.NN, cfg.C)
    idxsec = blob[cfg.OFF_IDX:cfg.OFF_ED3]
    ED3 = blob[cfg.OFF_ED3:cfg.OFF_WF1].view(np.float16).reshape(
        128, cfg.NCHT, 3)
    sigma = _host_prep_mesh(cfg, np.asarray(x, np.float32),
                            np.asarray(edge), np.asarray(pseudo, np.float32),
                            tab0c, idxsec, ED3)
    return blob, sigma


def _host_prep_mesh(cfg, x, edge, pseudo, tab0c, idxsec, ED3):
    """Vectorized per-mesh host preprocessing, writing into blob views:
    tab0c [NN,32] f16, idxsec [2*NSLOTT] i16, ED3 [128,NCHT,3] f16.
    Returns sigma (node->row permutation)."""
    npdt = _np_dt(cfg.DT)
    src = np.ascontiguousarray(edge[0], np.int32)
    dst = np.ascontiguousarray(edge[1], np.int32)
    E = src.shape[0]
    capa = cfg.NCHA * 128

    deg = np.bincount(dst, minlength=cfg.N)
    for attempt in range(2):
        if attempt == 0:
            sigma = _snake_sigma(deg, cfg.NWT, 128).astype(np.int32)
        else:
            sigma = _balance_nodes_greedy(deg, cfg.NWT).astype(np.int32)
        gsrc = sigma[src]
        gdst = sigma[dst]
        # key = window(dst)*2 + srcpass fits int16 (< 2*NWT = 784)
        key = ((gdst >> 7) * 2 + (gsrc >= cfg.NHALF)).astype(np.int16)
        order = np.argsort(key, kind="stable")   # radix on int16
        sk = key[order].astype(np.int32)
        starts = np.searchsorted(sk, np.arange(2 * cfg.NWT + 1)).astype(
            np.int32)
        slot = np.arange(E, dtype=np.int32) - starts[sk]
        if slot.max() < capa:
            break
    else:
        raise RuntimeError("window capacity exceeded")

    w = sk >> 1
    pp = sk & 1

    # per-slot edge data; precompute f16 source columns, then gather
    dlow = (gdst & 127).astype(npdt)
    ps16 = pseudo.astype(npdt)
    ED3[:, :, 0] = 128.0          # sentinel: empty slots match no node lane
    ED3[:, :, 1:] = 0
    ev = np.empty((E, 3), npdt)
    ev[:, 0] = dlow[order]
    ev[:, 1:] = ps16[order]
    ch = pp * cfg.NCHA + w * cfg.NCH + (slot >> 7)
    ED3[slot & 127, ch] = ev

    # gather indices: linear per-pass layout, then reshape-transpose into
    # the 16-partition-wrapped layout SWDGE wants
    gslot = w * capa + slot
    vals = (gsrc[order] - pp * cfg.NHALF).astype(np.int16)
    IDXL = np.zeros((2, cfg.NSLOTT), np.int16)
    IDXL[pp, gslot] = vals
    idxsec.reshape(2, 16, cfg.NI16)[:] = IDXL.reshape(
        2, cfg.NI16, 16).transpose(0, 2, 1)

    tab0c[:] = 0
    tab0c[sigma] = x.astype(npdt)
    return sigma


# ------------------------------------------------------------- bass program

def _ap(t, offset, pattern):
    base = t if isinstance(t, bass.AP) else t[:]
    return bass.AP(base.tensor, base.offset + offset, pattern)


def _sap(t, offset, freedims, npart=None):
    base = t if isinstance(t, bass.AP) else t[:]
    p = base.ap[0]
    part = [p[0], p[1] if npart is None else npart]
    return bass.AP(base.tensor, base.offset + offset, [part] + list(freedims))


def build_program(cfg, ncores):
    nc = bacc.Bacc("TRN2", target_bir_lowering=False, debug=False,
                   num_devices=ncores)
    DT = cfg.DT
    C, O, KK, RL = cfg.C, cfg.O, cfg.KK, cfg.RL
    NCH, NCHA, GW = cfg.NCH, cfg.NCHA, cfg.GW
    NI16, NCHT, NN, NHALF = cfg.NI16, cfg.NCHT, cfg.NN, cfg.NHALF
    NG = cfg.NWT // GW            # 98 groups cover the whole mesh
    UD = KK * C                   # 288

    blob = nc.dram_tensor("blob", [cfg.TOT16], I16, kind="ExternalInput")
    outt = nc.dram_tensor("out", [NN, C], DT, kind="ExternalOutput")

    bi16 = blob.ap()
    bf16 = bi16.bitcast(F16)

    def bview(base, off, pattern):
        return bass.AP(base.tensor, off, pattern)

    with tile.TileContext(nc, num_cores=ncores) as tc:
        with tc.tile_pool(name="const", bufs=1) as cpool, \
             tc.tile_pool(name="dram", bufs=1, space="DRAM") as dpool, \
             tc.tile_pool(name="work", bufs=2) as wpool, \
             tc.tile_pool(name="psum", bufs=1, space="PSUM") as ppool:

            from concourse import library_config
            nc.gpsimd.load_library(library_config.mlp)

            # ---- constants
            iotai = cpool.tile([128, 128], I32, name="iotai")
            nc.gpsimd.iota(iotai[:], pattern=[[1, 128]], base=0,
                           channel_multiplier=0)
            iotaf = cpool.tile([128, 128], DT, name="iotaf")
            nc.vector.tensor_copy(iotaf[:], iotai[:])
            idximp = cpool.tile([128, 128], I32, name="idximp")
            nc.gpsimd.iota(idximp[:], pattern=[[1, 128]], base=0,
                           channel_multiplier=-1)
            identf = cpool.tile([128, 128], F32, name="identf")
            nc.vector.tensor_scalar(out=identf[:], in0=idximp[:],
                                    scalar1=0, scalar2=None,
                                    op0=mybir.AluOpType.is_equal)
            identd = identf
            if DT != F32:
                identd = cpool.tile([128, 128], DT, name="identd")
                nc.vector.tensor_copy(identd[:], identf[:])

            # weights: f16 in the blob, converted to f32 tiles where the
            # node-side matmul accumulates in f32
            wfs, rts, bbs = [], [], []
            for i, (owf, ort, obb) in enumerate(
                    [(cfg.OFF_WF1, cfg.OFF_RT1, cfg.OFF_BB1),
                     (cfg.OFF_WF2, cfg.OFF_RT2, cfg.OFF_BB2)]):
                wf16 = cpool.tile([96, 96], DT, name=f"wf16_{i}")
                nc.sync.dma_start(
                    wf16[:], bview(bf16, owf, [[96, 96], [1, 96]]))
                wf32 = cpool.tile([96, 96], F32, name=f"wfs{i}")
                nc.vector.tensor_copy(wf32[:], wf16[:])
                wfs.append(wf32)
                rt = cpool.tile([C, O], DT, name=f"rts{i}")
                nc.sync.dma_start(
                    rt[:], bview(bf16, ort, [[O, C], [1, O]]))
                rts.append(rt)
                bb16 = cpool.tile([O, 1], DT, name=f"bb16_{i}")
                nc.sync.dma_start(
                    bb16[:], bview(bf16, obb, [[1, O], [1, 1]]))
                bb32 = cpool.tile([O, 1], F32, name=f"bbs{i}")
                nc.vector.tensor_copy(bb32[:], bb16[:])
                bbs.append(bb32)

            # ---- internal DRAM
            texp = dpool.tile([NN, RL], DT, name="texp")
            tfull = dpool.tile([NN, RL], DT, name="tfull")
            idxT = dpool.tile([128, 2 * NI16], I16, name="idxT")
            edd10 = dpool.tile([128, NCHT * 10], DT, name="edd10")

            # ---- prepass 1: expand the node table to the 256B granule
            nc.sync.dma_start(
                _ap(texp, 0, [[RL, NN], [1, C]]),
                bview(bf16, cfg.OFF_TAB, [[C, NN], [1, C]]))

            # ---- prepass 2: replicate gather indices x8 into 128 partitions
            for pp in range(2):
                for k in range(8):
                    nc.sync.dma_start(
                        _ap(idxT, k * 16 * (2 * NI16) + pp * NI16,
                            [[2 * NI16, 16], [1, NI16]]),
                        bview(bi16, cfg.OFF_IDX + pp * 16 * NI16,
                              [[NI16, 16], [1, NI16]]))

            # ---- prepass 3: expand (dstoff,t0,t1) -> (dstoff, 9 basis vals)
            SQH = float(np.sqrt(0.5))
            bias_sq = cpool.tile([128, 1], F32, name="bias_sq")
            nc.vector.memset(bias_sq[:], SQH)
            bias_z = cpool.tile([128, 1], F32, name="bias_z")
            nc.vector.memset(bias_z[:], 0.0)
            NCHUNK = 16
            CC = NCHT // NCHUNK
            for ci in range(NCHUNK):
                e3 = wpool.tile([128, CC * 3], DT, name="pe3", bufs=2)
                nc.sync.dma_start(
                    e3[:], bview(bf16, cfg.OFF_ED3 + ci * CC * 3,
                                 [[NCHT * 3, 128], [1, CC * 3]]))
                bts = []
                for d in range(2):
                    bt = wpool.tile([128, CC * 3], DT, name=f"pb{d}", bufs=2)
                    tv = _sap(e3, 1 + d, [[3, CC]])
                    # open quadratic B-spline pieces of t in [0,1):
                    # B0 = 0.5(1-t)^2, B2 = 0.5 t^2, B1 = 1 - B0 - B2
                    nc.scalar.activation(
                        _sap(bt, 0, [[3, CC]]), tv,
                        mybir.ActivationFunctionType.Square,
                        bias=bias_sq[:], scale=-SQH)
                    nc.scalar.activation(
                        _sap(bt, 2, [[3, CC]]), tv,
                        mybir.ActivationFunctionType.Square,
                        bias=bias_z[:], scale=SQH)
                    tmp = wpool.tile([128, CC], DT, name=f"pt{d}", bufs=2)
                    nc.vector.tensor_tensor(
                        out=tmp[:], in0=_sap(bt, 0, [[3, CC]]),
                        in1=_sap(bt, 2, [[3, CC]]),
                        op=mybir.AluOpType.add)
                    nc.vector.tensor_scalar(
                        out=_sap(bt, 1, [[3, CC]]), in0=tmp[:],
                        scalar1=-1.0, scalar2=1.0,
                        op0=mybir.AluOpType.mult,
                        op1=mybir.AluOpType.add)
                    bts.append(bt)
                e10 = wpool.tile([128, CC * 10], DT, name="pe10", bufs=2)
                nc.vector.tensor_copy(_sap(e10, 0, [[10, CC]]),
                                      _sap(e3, 0, [[3, CC]]))
                # basis[c, 3j+i] = B1[c,j] * B0[c,i]
                nc.vector.tensor_tensor(
                    out=_sap(e10, 1, [[10, CC], [3, 3], [1, 3]]),
                    in0=_sap(bts[0], 0, [[3, CC], [0, 3], [1, 3]]),
                    in1=_sap(bts[1], 0, [[3, CC], [1, 3], [0, 3]]),
                    op=mybir.AluOpType.mult)
                nc.sync.dma_start(
                    _ap(edd10, ci * CC * 10, [[NCHT * 10, 128], [1, CC * 10]]),
                    e10[:])

            nsg = GW * NCHA * 128          # gather idxs per (group, pass)
            npiece = nsg // cfg.NSPLIT
            nchp = GW * NCHA // cfg.NSPLIT  # gathered chunks per piece

            def layer(tabsrc, xown, xrl, wfsb, rtsb, bbsb, rows_out, orl,
                      last):
                for g0 in range(0, NG, cfg.SG):
                    gcnt = min(cfg.SG, NG - g0)
                    sidxt = [None, None]
                    for pdx in range(2):
                        st = wpool.tile([128, cfg.SG * nsg // 16], I16,
                                        name=f"sidx{pdx}", bufs=2)
                        nc.sync.dma_start(
                            _sap(st, 0, [[1, gcnt * nsg // 16]]),
                            _ap(idxT, pdx * NI16 + g0 * nsg // 16,
                                [[2 * NI16, 128], [1, gcnt * nsg // 16]]))
                        sidxt[pdx] = st
                    for gl in range(gcnt):
                        g = g0 + gl
                        xjt = []
                        for pdx in range(2):
                            xj = wpool.tile([128, GW * NCHA * RL], DT,
                                            name=f"xj{pdx}", bufs=2)
                            for s in range(cfg.NSPLIT):
                                nc.gpsimd.dma_gather(
                                    out_ap=_sap(xj, s * nchp * RL,
                                                [[RL, nchp], [1, RL]]),
                                    in_ap=_ap(tabsrc, pdx * NHALF * RL,
                                              [[RL, NHALF], [1, RL]]),
                                    idxs_ap=_sap(
                                        sidxt[pdx],
                                        (gl * nsg + s * npiece) // 16,
                                        [[1, npiece // 16]]),
                                    num_idxs=npiece,
                                    num_idxs_reg=npiece,
                                    elem_size=RL,
                                    single_packet=False,
                                )
                            xjt.append(xj)
                        edt = wpool.tile([128, GW * NCH * 10], DT,
                                         name="edt", bufs=2)
                        nc.sync.dma_start(
                            edt[:],
                            _ap(edd10, g * GW * NCH * 10,
                                [[NCHT * 10, 128], [1, GW * NCH * 10]]))
                        xwing = wpool.tile([128, GW * C], DT, name="xwing",
                                           bufs=2)
                        nc.sync.dma_start(
                            xwing[:],
                            _ap(xown, g * GW * 128 * xrl,
                                [[xrl, 128], [128 * xrl, GW], [1, C]]))
                        rowsg = wpool.tile([128, GW * C], DT, name="rowsg",
                                           bufs=2)

                        for wl in range(GW):
                            u = wpool.tile([128, NCH * UD], DT, name="u",
                                           bufs=2)
                            for pdx in range(2):
                                nc.vector.tensor_tensor(
                                    out=_sap(u, pdx * NCHA * UD,
                                             [[UD, NCHA], [C, KK], [1, C]]),
                                    in0=_sap(xjt[pdx], wl * NCHA * RL,
                                             [[RL, NCHA], [0, KK], [1, C]]),
                                    in1=_sap(edt,
                                             (wl * NCH + pdx * NCHA) * 10 + 1,
                                             [[10, NCHA], [1, KK], [0, C]]),
                                    op=mybir.AluOpType.mult)
                            inc = wpool.tile([128, NCH * 128], DT,
                                             name="inc", bufs=2)
                            nc.vector.tensor_tensor(
                                out=_sap(inc, 0, [[128, NCH], [1, 128]]),
                                in0=_sap(iotaf, 0, [[0, NCH], [1, 128]]),
                                in1=_sap(edt, wl * NCH * 10,
                                         [[10, NCH], [0, 128]]),
                                op=mybir.AluOpType.is_equal)

                            z = ppool.tile([128, UD], F32, name="z", bufs=2)
                            for c in range(NCH):
                                nc.tensor.matmul(
                                    z[:],
                                    _sap(inc, c * 128, [[1, 128]]),
                                    _sap(u, c * UD, [[1, UD]]),
                                    start=(c == 0), stop=(c == NCH - 1))

                            zsb = wpool.tile([128, UD], F32, name="zsb",
                                             bufs=2)
                            nc.scalar.copy(zsb[:], z[:])
                            zt = ppool.tile([96, 384], F32, name="zt",
                                            bufs=2)
                            for j in range(3):
                                nc.tensor.transpose(
                                    _sap(zt, j * 128, [[1, 128]]),
                                    _sap(zsb, j * 96, [[1, 96]]),
                                    identf[:])
                            ztsb = wpool.tile([96, 384], F32, name="ztsb",
                                              bufs=2)
                            nc.scalar.copy(ztsb[:], zt[:])

                            agg = ppool.tile([O, 128], F32, name="agg",
                                             bufs=1)
                            for j in range(3):
                                nc.tensor.matmul(
                                    agg[:],
                                    _sap(wfsb, j * 32, [[1, 32]]),
                                    _sap(ztsb, j * 128, [[1, 128]]),
                                    start=(j == 0), stop=False)
                            xt = ppool.tile([C, 128], DT, name="xt", bufs=1)
                            nc.tensor.transpose(
                                xt[:],
                                _sap(xwing, wl * C, [[1, C]]),
                                identd[:])
                            xtsb = wpool.tile([C, 128], DT, name="xtsb",
                                              bufs=2)
                            nc.scalar.copy(xtsb[:], xt[:])
                            nc.tensor.matmul(agg[:], rtsb[:], xtsb[:],
                                             start=False, stop=True)
                            ht = wpool.tile([O, 128], DT, name="ht",
                                            bufs=2)
                            nc.scalar.activation(
                                ht[:], agg[:],
                                mybir.ActivationFunctionType.Relu,
                                bias=bbsb[:], scale=1.0)
                            rows = ppool.tile([128, O], DT, name="rows",
                                              bufs=1)
                            nc.tensor.transpose(
                                rows[:], ht[:],
                                _sap(identd, 0, [[1, 32]], npart=32))
                            nc.scalar.copy(
                                _sap(rowsg, wl * C, [[1, C]]), rows[:])

                        nc.sync.dma_start(
                            _ap(rows_out, g * GW * 128 * orl,
                                [[orl, 128], [128 * orl, GW], [1, C]]),
                            rowsg[:])

            # layer 1: gathers from texp, root term from the compact blob
            # table, writes the local full table
            layer(texp[:], bview(bf16, cfg.OFF_TAB, [[1, 1]]), C,
                  wfs[0], rts[0], bbs[0], tfull[:], RL, last=False)
            # layer 2: gathers from tfull, writes the compact f16 output
            layer(tfull[:], tfull[:], RL, wfs[1], rts[1], bbs[1],
                  outt.ap(), C, last=True)

    nc.finalize()
    return nc


# ------------------------------------------------------------------- runner

_RT = None


def _get_runtime(cfg):
    global _RT
    if _RT is not None:
        return _RT

    import jax
    import jax.numpy as jnp
    from jax.sharding import Mesh, PartitionSpec, NamedSharding
    from jax.experimental.shard_map import shard_map
    from concourse.bass2jax import (_bass_exec_p, install_neuronx_cc_hook,
                                    partition_id_tensor)

    try:
        jax.config.update("jax_compilation_cache_dir", "/tmp/meshconv_jaxcache")
        jax.config.update("jax_persistent_cache_min_compile_time_secs", 0.5)
    except Exception:
        pass

    nc = build_program(cfg, cfg.NCORES)
    install_neuronx_cc_hook()

    partition_name = (nc.partition_id_tensor.name
                      if nc.partition_id_tensor else None)
    in_names, out_names, out_avals = [], [], []
    for alloc in nc.m.functions[0].allocations:
        if not isinstance(alloc, mybir.MemoryLocationSet):
            continue
        name = alloc.memorylocations[0].name
        if alloc.kind == "ExternalInput":
            if name != partition_name:
                in_names.append(name)
        elif alloc.kind == "ExternalOutput":
            out_names.append(name)
            out_avals.append(jax.core.ShapedArray(
                tuple(alloc.tensor_shape), mybir.dt.np(alloc.dtype)))
    n_params = len(in_names)
    n_outs = len(out_names)
    all_names = list(in_names) + list(out_names)
    if partition_name is not None:
        all_names.append(partition_name)
    donate = tuple(range(n_params, n_params + n_outs))

    n_cores = cfg.NCORES
    devices = jax.devices()[:n_cores]
    mesh = Mesh(np.asarray(devices), ("core",))
    spec = NamedSharding(mesh, PartitionSpec("core"))

    def _body(*args):
        operands = list(args)
        if partition_name is not None:
            operands.append(partition_id_tensor())
        outs = _bass_exec_p.bind(
            *operands,
            out_avals=tuple(out_avals),
            in_names=tuple(all_names),
            out_names=tuple(out_names),
            lowering_input_output_aliases=(),
            sim_require_finite=True,
            sim_require_nnan=True,
            nc=nc,
        )
        return tuple(outs)

    sharded = jax.jit(
        shard_map(_body, mesh=mesh,
                  in_specs=(PartitionSpec("core"),) * (n_params + n_outs),
                  out_specs=(PartitionSpec("core"),) * n_outs,
                  check_rep=False),
        donate_argnums=donate, keep_unused=True)

    def _zeros():
        return tuple(
            jnp.zeros((n_cores * a.shape[0], *a.shape[1:]), a.dtype)
            for a in out_avals)
    zeros_fn = jax.jit(_zeros, out_shardings=(spec,) * n_outs)

    _RT = dict(nc=nc, jax=jax, mesh=mesh, spec=spec, devices=devices,
               in_names=in_names, out_names=out_names, out_avals=out_avals,
               sharded=sharded, zeros_fn=zeros_fn)
    return _RT


def _const16(cfg, W1, root1, b1, W2, root2, b2):
    """Pack weights into the int16 blob tail (f16 payloads)."""
    sec = np.zeros(cfg.TOT16 - cfg.OFF_WF1, np.int16)

    def put(off, arr16):
        v = arr16.view(np.int16).ravel()
        sec[off - cfg.OFF_WF1:off - cfg.OFF_WF1 + v.size] = v

    for W, off in ((W1, cfg.OFF_WF1), (W2, cfg.OFF_WF2)):
        Wflat = np.asarray(W, np.float32).reshape(cfg.KK * cfg.C, cfg.O)
        wfl = np.zeros((96, 96), np.float32)
        for j in range(3):
            wfl[:, 32 * j:32 * j + 32] = Wflat[96 * j:96 * j + 96, :]
        put(off, wfl.astype(np.float16))
    put(cfg.OFF_RT1, np.asarray(root1, np.float32).astype(np.float16))
    put(cfg.OFF_RT2, np.asarray(root2, np.float32).astype(np.float16))
    put(cfg.OFF_BB1, np.asarray(b1, np.float32).astype(np.float16))
    put(cfg.OFF_BB2, np.asarray(b2, np.float32).astype(np.float16))
    return sec


def run(cfg, images, edges, pseudo, W1, root1, b1, W2, root2, b2,
        trace=False, trace_out=None):
    rt = _get_runtime(cfg)
    jax = rt["jax"]
    devices = rt["devices"]

    csec = _const16(cfg, W1, root1, b1, W2, root2, b2)

    # host prep per mesh straight into the blob; device_put (async) each
    # blob as soon as it is ready so uploads overlap later meshes' prep
    shards = [None] * cfg.NCORES
    sigmas = [None] * cfg.B
    for b in range(cfg.B):
        blob = np.empty(cfg.TOT16, np.int16)
        tab0c = blob[cfg.OFF_TAB:cfg.OFF_IDX].view(np.float16).reshape(
            cfg.NN, cfg.C)
        idxsec = blob[cfg.OFF_IDX:cfg.OFF_ED3]
        ED3 = blob[cfg.OFF_ED3:cfg.OFF_WF1].view(np.float16).reshape(
            128, cfg.NCHT, 3)
        sigma = _host_prep_mesh(
            cfg, np.asarray(images[b], np.float32),
            np.asarray(edges[b]), np.asarray(pseudo[b], np.float32),
            tab0c, idxsec, ED3)
        blob[cfg.OFF_WF1:] = csec
        sigmas[b] = sigma
        shards[b] = jax.device_put(blob, devices[b])

    garr = jax.make_array_from_single_device_arrays(
        (cfg.NCORES * cfg.TOT16,), rt["spec"], shards)
    zeros = rt["zeros_fn"]()

    out_arrs = rt["sharded"](garr, *zeros)
    outg = np.asarray(out_arrs[0]).reshape(cfg.NCORES, cfg.NN, cfg.C)

    out = np.empty((cfg.B, cfg.N, cfg.O), np.float32)
    for b in range(cfg.B):
        out[b] = outg[b].astype(np.float32)[sigmas[b]]
    return out


def kernel(images, edges, pseudo, W1, root1, b1, W2, root2, b2):
    cfg = CFG()
    return run(cfg, images, edges, pseudo, W1, root1, b1,
               W2, root2, b2)
